# revision 61
# baseline (speedup 1.0000x reference)
"""Causal multi-head self-attention on 8 Trainium2 NeuronCores.

B=2, N=2048, D=1024, H=16 heads of d=64. Head-parallel sharding: core c
owns heads 2c, 2c+1. Each core reads the full (transposed, bf16) X and its
128-column slice of Wq/Wk/Wv (and 128-row slice of Wo), computes
Q^T/K^T/V for its 2 heads, runs causal flash-style attention entirely in
"transposed" layout (zero on-device transposes), applies its Wo slice, and
writes a full-shape partial output. The host sums the 8 partials + bo.

Per-core dataflow (all matmuls bf16 with fp32 PSUM accumulation):
  X^T (host-transposed, bf16)  --DMA-->  SBUF, 8 k-tiles [128, 4096]
  Q^T = Wq_c^T X^T (pre-scaled by 1/sqrt(dk) on host)   [128, 4096]
  K^T = Wk_c^T X^T                                      [128, 4096]
  V   = X Wv_c (normal layout, 32 blocks [128 n, 128 dv], stored with
        interleaved ones-columns for the sum-exp trick)
  per (batch, q-chunk of 512, k-block of 128):
    S^T[nk, nq] = (K^T slice).T @ (Q^T slice) for BOTH heads into one
      [128, 1024] PSUM tile (2 banks), so a single ACT exp instruction
      covers both heads.
    E = exp(S^T)  (no max-subtraction; scores are O(1))  -> SBUF bf16
    causal: skip k-blocks above the diagonal, mask the diagonal 128x128
    AV^T[dv, nq] (+ sumexp row via ones-column in V)  accumulated in PSUM
  The AV matmul for block i is emitted AFTER the S matmul for block i+1
  (1-deep software pipeline) so the PE never sits waiting on ACT's exp:
  exp(i) runs on ACT while PE does AV(i-1) and S(i+1).
  normalize per batch: AV^T * broadcast(1/sumexp), heads stacked [128, n]
  O_partial = (AVn^T).T @ Wo_c  -> DMA to DRAM
"""

import numpy as np

B, N, D, H, DK, DV = 2, 2048, 1024, 16, 64, 64
NCORES = 8
HPC = H // NCORES  # heads per core = 2
BN = B * N  # 4096
NQ_CHUNK = 512  # query chunk (psum free dim)
NK_BLK = 128  # key block (psum partition dim)
N_JCH = N // NQ_CHUNK  # 4 q-chunks per batch
N_KBLK = N // NK_BLK  # 16 k-blocks per batch
KT_PER_D = D // 128  # 8 contraction tiles for the projections
NBLK_ALL = BN // NK_BLK  # 32 n-blocks over both batches

_STATE = {}


def _build_nc(iters=1, phase="full"):
    import contextlib

    import concourse.bacc as bacc
    import concourse.mybir as mybir
    import concourse.tile as tile
    from concourse.masks import make_upper_triangular

    f32 = mybir.dt.float32
    bf16 = mybir.dt.bfloat16
    AF = mybir.ActivationFunctionType

    nc = bacc.Bacc("TRN2", target_bir_lowering=False, debug=False)

    xt_d = nc.dram_tensor("xt", [D, BN], bf16, kind="ExternalInput")
    wq_d = nc.dram_tensor("wq", [128, KT_PER_D, 128], bf16, kind="ExternalInput")
    wk_d = nc.dram_tensor("wk", [128, KT_PER_D, 128], bf16, kind="ExternalInput")
    wv_d = nc.dram_tensor("wv", [128, KT_PER_D, 128], bf16, kind="ExternalInput")
    wo_d = nc.dram_tensor("wo", [128, D], bf16, kind="ExternalInput")
    bq_d = nc.dram_tensor("bq", [128, 1], f32, kind="ExternalInput")
    bk_d = nc.dram_tensor("bk", [128, 1], f32, kind="ExternalInput")
    bv_d = nc.dram_tensor("bv", [128, 1], f32, kind="ExternalInput")
    out_d = nc.dram_tensor("out", [BN, D], bf16, kind="ExternalOutput")

    with tile.TileContext(nc) as tc:
        with (
            tc.tile_pool(name="const", bufs=1) as const,
            tc.tile_pool(name="xtp", bufs=KT_PER_D) as xtp,  # bufs per col-group tag
            tc.tile_pool(name="persist", bufs=1) as persist,
            tc.tile_pool(name="avn", bufs=3) as avnp,
            tc.tile_pool(name="expp", bufs=6) as expp,
            tc.tile_pool(name="s0p", bufs=3) as s0p,
            tc.tile_pool(name="bcp", bufs=3) as bcp,
            tc.tile_pool(name="h1tp", bufs=2) as h1tp,
            tc.tile_pool(name="osb", bufs=3) as osbp,
        ):
            # ---- constants ----
            wq_sb = const.tile([128, KT_PER_D, 128], bf16, tag="wq")
            wk_sb = const.tile([128, KT_PER_D, 128], bf16, tag="wk")
            wv_sb = const.tile([128, KT_PER_D, 128], bf16, tag="wv")
            nc.sync.dma_start(wq_sb[:], wq_d[:])
            nc.sync.dma_start(wk_sb[:], wk_d[:])
            nc.sync.dma_start(wv_sb[:], wv_d[:])
            wo_sb = const.tile([128, D], bf16, tag="wo")
            nc.sync.dma_start(wo_sb[:], wo_d[:])
            bq_sb = const.tile([128, 1], f32, tag="bq")
            bk_sb = const.tile([128, 1], f32, tag="bk")
            nc.sync.dma_start(bq_sb[:], bq_d[:])
            nc.sync.dma_start(bk_sb[:], bk_d[:])
            bv_sb = const.tile([128, 1], f32, tag="bv")
            nc.sync.dma_start(bv_sb[:], bv_d[:])
            # causal keep-mask: mask[p, f] = 1.0 iff f >= p
            mask = const.tile([128, 128], bf16, tag="mask")
            make_upper_triangular(nc, mask[:], val=1.0, diag=True)

            # ---- X^T tiles, split by column-group so matmuls start on the
            # first 256 KB instead of after the full 8 MB ----
            NCG = 4
            CGW = BN // NCG  # 1024 cols per group
            xt_t = {}
            for cg in range(NCG):
                for k in range(KT_PER_D):
                    t = xtp.tile([128, CGW], bf16, tag=f"xt{cg}", name=f"xt{k}_{cg}")
                    nc.sync.dma_start(
                        t[:], xt_d[k * 128 : (k + 1) * 128, cg * CGW : (cg + 1) * CGW]
                    )
                    xt_t[(k, cg)] = t

            def xt_slice(k, c0, c1):
                cg = c0 // CGW
                assert c1 <= (cg + 1) * CGW
                return xt_t[(k, cg)][:, c0 - cg * CGW : c1 - cg * CGW]

            # ---- persistent activations ----
            QT = persist.tile([128, BN], bf16, tag="qt")
            KT = persist.tile([128, BN], bf16, tag="kt")
            VT = persist.tile([128, BN], bf16, tag="vt")
            # V blocks with interleaved ones cols: [V_h0 | 1 | V_h1 | 1]
            V_sb = persist.tile([128, NBLK_ALL, 130], bf16, tag="vsb")
            vview = V_sb.rearrange("p i (g c) -> p i g c", c=65)
            nc.vector.memset(vview[:, :, :, 64:65], 1.0)

            def body():
                # PSUM budget (8 banks): s 2x2 + av 2 + pq (proj/out shared) 2
                with (
                    tc.tile_pool(name="pp", bufs=2, space="PSUM") as pp,
                    tc.tile_pool(name="sp", bufs=2, space="PSUM") as sp,
                    tc.tile_pool(name="avp", bufs=2, space="PSUM") as avp,
                ):
                    def emit_proj_chunk(j8, pump=False, which=("q", "k", "v"), evac_act=False):
                        """Q/K/V^T projection for one 512-col chunk; as a
                        generator it yields once per full 8-matmul chain so
                        pumped fillers never get split mid-chain (split
                        chains run ~1.7x slower on HW)."""
                        c0, c1 = j8 * NQ_CHUNK, (j8 + 1) * NQ_CHUNK
                        for w_sb, b_sb, dst, nm in (
                            (wq_sb, bq_sb, QT, "q"),
                            (wk_sb, bk_sb, KT, "k"),
                            (wv_sb, bv_sb, VT, "v"),
                        ):
                            if nm not in which:
                                continue
                            ps = pp.tile(
                                [128, NQ_CHUNK], f32, tag="pq", name=f"ps{nm}{j8}"
                            )
                            for k in range(KT_PER_D):
                                nc.tensor.matmul(
                                    ps[:],
                                    w_sb[:, k, :],
                                    xt_slice(k, c0, c1),
                                    start=(k == 0),
                                    stop=(k == KT_PER_D - 1),
                                )
                            if evac_act:
                                # fill idle ACT at kernel start (before exps)
                                nc.scalar.activation(
                                    dst[:, c0:c1], ps[:], AF.Identity, bias=b_sb[:, 0:1]
                                )
                            else:
                                nc.vector.tensor_scalar_add(dst[:, c0:c1], ps[:], b_sb[:, 0:1])
                            if pump:
                                yield

                    def gen_v_chunk(j8):
                        yield from emit_proj_chunk(j8, pump=True, which=("v",))
                        emit_v_layout(0, j8)
                        yield

                    def emit_v_layout(b, q):
                        # one 512-col quarter: xbar transpose to contiguous
                        # scratch (strided 3D transpose output misbehaves on
                        # HW), then DVE-copy into the interleaved layout.
                        nb0 = b * N_KBLK + q * 4
                        for h in range(HPC):
                            vtmp = h1tp.tile(
                                [128, 4, 64], bf16, tag="vtmp", name=f"vtmp{b}_{q}_{h}"
                            )
                            nc.sync.dma_start_transpose(
                                vtmp[:],
                                VT[h * 64 : (h + 1) * 64, b * N + q * 512 : b * N + (q + 1) * 512],
                            )
                            nc.vector.tensor_copy(
                                V_sb[:, nb0 : nb0 + 4, 65 * h : 65 * h + 64], vtmp[:]
                            )

                    def gen_b1_tail():
                        for j8 in range(N_JCH, 2 * N_JCH):
                            yield from emit_proj_chunk(j8, pump=True)
                            emit_v_layout(1, j8 - N_JCH)
                            yield

                    def gen_out(b, j, AVnj):
                        """Output projection for one normalized q-chunk."""
                        for nb in range(NQ_CHUNK // 128):
                            row0 = b * N + j * NQ_CHUNK + nb * 128
                            osb = osbp.tile(
                                [128, D], bf16, tag="osb", name=f"osb{b}_{j}_{nb}"
                            )
                            for half in range(2):
                                o_ps = pp.tile(
                                    [128, 512], f32, tag="pq", name=f"o{b}_{j}_{nb}_{half}"
                                )
                                nc.tensor.matmul(
                                    o_ps[:],
                                    AVnj[:, nb * 128 : (nb + 1) * 128],
                                    wo_sb[:, half * 512 : (half + 1) * 512],
                                    start=True,
                                    stop=True,
                                )
                                nc.vector.tensor_copy(
                                    osb[:, half * 512 : (half + 1) * 512], o_ps[:]
                                )
                                yield
                            nc.sync.dma_start(out_d[row0 : row0 + 128, :], osb[:])
                            yield

                    pending = []

                    def pump(n):
                        done = 0
                        while pending and done < n:
                            try:
                                next(pending[0])
                                done += 1
                            except StopIteration:
                                pending.pop(0)

                    def drain(gen):
                        """Finish emitting one specific pending generator."""
                        while gen in pending:
                            try:
                                next(gen)
                            except StopIteration:
                                pending.remove(gen)

                    b1g = gen_b1_tail()
                    pending.append(b1g)



                    def emit_S(b, j, i):
                        """Both heads' S^T for k-block i into one 2-bank psum
                        tile: h0 cols [f0,512), h1 cols [512+f0, 1024)."""
                        cb = b * N
                        q0 = cb + j * NQ_CHUNK
                        r = i - j * (NQ_CHUNK // NK_BLK)
                        f0 = 128 * r if r > 0 else 0
                        s_ps = sp.tile(
                            [128, 2 * NQ_CHUNK], f32, tag="s", name=f"s{b}_{j}_{i}"
                        )
                        for h in range(HPC):
                            nc.tensor.matmul(
                                s_ps[:, h * NQ_CHUNK + f0 : (h + 1) * NQ_CHUNK],
                                KT[h * 64 : (h + 1) * 64, cb + i * 128 : cb + (i + 1) * 128],
                                QT[h * 64 : (h + 1) * 64, q0 + f0 : q0 + NQ_CHUNK],
                                start=True,
                                stop=True,
                            )
                        return s_ps, f0, r

                    def emit_exp(b, j, i, s_ps, f0, r):
                        """One ACT exp instruction covering both heads."""
                        et = expp.tile(
                            [128, 2 * NQ_CHUNK], bf16, tag="exp", name=f"e{b}_{j}_{i}"
                        )
                        if f0 == 0:
                            nc.scalar.activation(et[:], s_ps[:], AF.Exp)
                        else:
                            sv = s_ps.rearrange("p (g c) -> p g c", c=NQ_CHUNK)
                            ev = et.rearrange("p (g c) -> p g c", c=NQ_CHUNK)
                            nc.scalar.activation(
                                ev[:, :, f0:NQ_CHUNK], sv[:, :, f0:NQ_CHUNK], AF.Exp
                            )
                        if r >= 0:
                            for h in range(HPC):
                                nc.vector.tensor_mul(
                                    et[:, h * NQ_CHUNK + f0 : h * NQ_CHUNK + f0 + 128],
                                    et[:, h * NQ_CHUNK + f0 : h * NQ_CHUNK + f0 + 128],
                                    mask[:],
                                )
                        return et

                    def emit_AV1(b, j, i, et, f0, h, av_ps, kmax):
                        nc.tensor.matmul(
                            av_ps[h][:, f0:NQ_CHUNK],
                            V_sb[:, b * N_KBLK + i, 65 * h : 65 * (h + 1)],
                            et[:, h * NQ_CHUNK + f0 : (h + 1) * NQ_CHUNK],
                            start=(i == 0),
                            stop=(i == kmax - 1),
                            skip_group_check=True,
                        )

                    if phase == "proj":
                        # projections only: all 8 chunks + V layouts
                        for j8 in range(N_JCH):
                            for _ in emit_proj_chunk(j8, pump=False,
                                                     evac_act=(j8 == 0)):
                                pass
                            emit_v_layout(0, j8)
                        pump(10**9)
                        nc.sync.dma_start(out_d[0:128, :], QT[:, 0:D])
                        return
                    for b in range(B):
                        j_order = range(N_JCH) if b == 0 else range(N_JCH - 1, -1, -1)
                        for j in j_order:
                            if b == 0:
                                # Q/K land just-in-time per chunk; V of chunk j
                                # becomes front-of-queue PE filler
                                qk = ("q", "k", "v") if j == 0 else ("q", "k")
                                for _ in emit_proj_chunk(j, pump=False, which=qk,
                                                         evac_act=(j == 0)):
                                    pass
                                if j == 0:
                                    emit_v_layout(0, 0)
                                else:
                                    # must fully emit before the i-loop: AV(i)
                                    # reads V_sb blocks this generator writes,
                                    # and a read emitted first gets no dep.
                                    # 2 extra pumps beyond the 2-step
                                    # generator fill the chunk-boundary
                                    # av-WAR window (measured faster)
                                    pending.insert(0, gen_v_chunk(j))
                                    pump(4)
                            else:
                                # same boundary treatment for b1 chunks:
                                # fill the av-WAR window with fillers
                                pump(4)
                            kmax = (j + 1) * (NQ_CHUNK // NK_BLK)
                            av_ps = [
                                avp.tile([65, NQ_CHUNK], f32, tag="av", name=f"av{b}_{j}_{h}")
                                for h in range(HPC)
                            ]
                            # pair-pipelined: emit S for blocks (2p, 2p+1),
                            # then the PREVIOUS pair's AVs ordered h-major so
                            # each head's two AVs chain in one PSUM group
                            # (measured 32% faster than block-major order)
                            prev = None
                            for p in range(kmax // 2):
                                cur = []
                                for i in (2 * p, 2 * p + 1):
                                    s_ps, f0, r = emit_S(b, j, i)
                                    cur.append((i, s_ps, f0, r))
                                if prev is not None:
                                    for h in range(HPC):
                                        for i, et, f0 in prev:
                                            emit_AV1(b, j, i, et, f0, h, av_ps, kmax)
                                prev = [
                                    (i, emit_exp(b, j, i, s_ps, f0, r), f0)
                                    for i, s_ps, f0, r in cur
                                ]
                                pump(2 if p == 0 else 1)
                            for h in range(HPC):
                                for i, et, f0 in prev:
                                    emit_AV1(b, j, i, et, f0, h, av_ps, kmax)
                            # normalize this chunk straight out of PSUM
                            AVnj = avnp.tile(
                                [128, NQ_CHUNK], bf16, tag="avn", name=f"avn{b}_{j}"
                            )
                            for h in range(HPC):
                                rc = s0p.tile(
                                    [65, NQ_CHUNK], f32, tag="rc", name=f"rc{b}_{j}_{h}"
                                )
                                nc.vector.reciprocal(rc[64:65, :], av_ps[h][64:65, :])
                                s0 = s0p.tile(
                                    [1, NQ_CHUNK], f32, tag="s0", name=f"s0_{b}_{j}_{h}"
                                )
                                # ACT-issued (HWDGE): skips the SP queue's
                                # out-DMA/transpose backlog at boundaries
                                nc.scalar.dma_start(s0[0:1, :], rc[64:65, :])
                                bc = bcp.tile(
                                    [64, NQ_CHUNK], f32, tag="bc", name=f"bc{b}_{j}_{h}"
                                )
                                nc.gpsimd.partition_broadcast(bc[:], s0[0:1, :])
                                if h == 0:
                                    nc.vector.tensor_mul(
                                        AVnj[0:64, :], av_ps[h][0:64, :], bc[:]
                                    )
                                else:
                                    h1t = h1tp.tile(
                                        [64, NQ_CHUNK], bf16, tag="h1t", name=f"h1t{b}_{j}"
                                    )
                                    nc.vector.tensor_mul(h1t[:], av_ps[h][0:64, :], bc[:])
                                    nc.scalar.dma_start(AVnj[64:128, :], h1t[:])
                            if phase == "noout":
                                nc.sync.dma_start(
                                    out_d[(b * N_JCH + j) * 128 : (b * N_JCH + j) * 128 + 128, 0:NQ_CHUNK],
                                    AVnj[:],
                                )
                            else:
                                pending.append(gen_out(b, j, AVnj))
                        if b == 0:
                            # batch-1 projections must be EMITTED before its
                            # attention reads QT/KT/V (emission-order dep
                            # safety) — but leftover out-proj fillers stay
                            # queued for b1's pair pumps instead of being
                            # dumped here as one ACT-idling PE backlog
                            drain(b1g)
                    pump(10**9)

            if iters > 1:
                with tc.For_i(0, iters, 1):
                    body()
            else:
                body()

    nc.compile()
    return nc


def _prep_in_maps(X, Wq, bq, Wk, bk, Wv, bv, Wo, bo):
    import ml_dtypes

    bf16 = ml_dtypes.bfloat16

    def _pkm(w):  # [D, 128] -> [128 partition, k, 128] tile layout
        return np.ascontiguousarray(
            w.reshape(KT_PER_D, 128, 128).transpose(1, 0, 2)
        ).astype(bf16)
    scale = np.float32(1.0 / np.sqrt(DK))
    Xf = np.asarray(X, dtype=np.float32).reshape(BN, D)
    xt = np.ascontiguousarray(Xf.T).astype(bf16)
    in_maps = []
    for c in range(NCORES):
        s = slice(c * 128, (c + 1) * 128)
        in_maps.append(
            {
                "xt": xt,
                "wq": _pkm(np.asarray(Wq, np.float32)[:, s] * scale),
                "wk": _pkm(np.asarray(Wk, np.float32)[:, s]),
                "wv": _pkm(np.asarray(Wv, np.float32)[:, s]),
                "wo": np.ascontiguousarray(np.asarray(Wo, np.float32)[s, :]).astype(bf16),
                "bq": np.ascontiguousarray(
                    (np.asarray(bq, np.float32)[s] * scale).reshape(128, 1)
                ),
                "bk": np.ascontiguousarray(np.asarray(bk, np.float32)[s].reshape(128, 1)),
                "bv": np.ascontiguousarray(np.asarray(bv, np.float32)[s].reshape(128, 1)),
            }
        )
    return in_maps


def _get_nc(iters=1, phase="full"):
    key = ("nc", iters, phase)
    if key not in _STATE:
        _STATE[key] = _build_nc(iters, phase)
    return _STATE[key]


def kernel(**inputs) -> np.ndarray:
    from concourse import bass_utils

    nc = _get_nc()
    in_maps = _prep_in_maps(**inputs)
    res = bass_utils.run_bass_kernel_spmd(nc, in_maps, core_ids=list(range(NCORES)))
    acc = np.zeros((BN, D), dtype=np.float32)
    for r in res.results:
        acc += np.asarray(r["out"], dtype=np.float32)
    acc += np.asarray(inputs["bo"], np.float32)[None, :]
    return acc.reshape(B, N, D)


# revision 63
# speedup vs baseline: 1.0226x; 1.0226x over previous
"""Causal multi-head self-attention on 8 Trainium2 NeuronCores.

B=2, N=2048, D=1024, H=16 heads of d=64. Head-parallel sharding: core c
owns heads 2c, 2c+1. Each core reads the full (transposed, bf16) X and its
128-column slice of Wq/Wk/Wv (and 128-row slice of Wo), computes
Q^T/K^T/V for its 2 heads, runs causal flash-style attention entirely in
"transposed" layout (zero on-device transposes), applies its Wo slice, and
writes a full-shape partial output. The host sums the 8 partials + bo.

Per-core dataflow (all matmuls bf16 with fp32 PSUM accumulation):
  X^T (host-transposed, bf16)  --DMA-->  SBUF, 8 k-tiles [128, 4096]
  Q^T = Wq_c^T X^T (pre-scaled by 1/sqrt(dk) on host)   [128, 4096]
  K^T = Wk_c^T X^T                                      [128, 4096]
  V   = X Wv_c (normal layout, 32 blocks [128 n, 128 dv], stored with
        interleaved ones-columns for the sum-exp trick)
  per (batch, q-chunk of 512, k-block of 128):
    S^T[nk, nq] = (K^T slice).T @ (Q^T slice) for BOTH heads into one
      [128, 1024] PSUM tile (2 banks), so a single ACT exp instruction
      covers both heads.
    E = exp(S^T)  (no max-subtraction; scores are O(1))  -> SBUF bf16
    causal: skip k-blocks above the diagonal, mask the diagonal 128x128
    AV^T[dv, nq] (+ sumexp row via ones-column in V)  accumulated in PSUM
  The AV matmul for block i is emitted AFTER the S matmul for block i+1
  (1-deep software pipeline) so the PE never sits waiting on ACT's exp:
  exp(i) runs on ACT while PE does AV(i-1) and S(i+1).
  normalize per batch: AV^T * broadcast(1/sumexp), heads stacked [128, n]
  O_partial = (AVn^T).T @ Wo_c  -> DMA to DRAM
"""

import numpy as np

B, N, D, H, DK, DV = 2, 2048, 1024, 16, 64, 64
NCORES = 8
HPC = H // NCORES  # heads per core = 2
BN = B * N  # 4096
NQ_CHUNK = 512  # query chunk (psum free dim)
NK_BLK = 128  # key block (psum partition dim)
N_JCH = N // NQ_CHUNK  # 4 q-chunks per batch
N_KBLK = N // NK_BLK  # 16 k-blocks per batch
KT_PER_D = D // 128  # 8 contraction tiles for the projections
NBLK_ALL = BN // NK_BLK  # 32 n-blocks over both batches

_STATE = {}


def _build_nc(iters=1, phase="full"):
    import contextlib

    import concourse.bacc as bacc
    import concourse.mybir as mybir
    import concourse.tile as tile
    from concourse.masks import make_upper_triangular

    f32 = mybir.dt.float32
    bf16 = mybir.dt.bfloat16
    AF = mybir.ActivationFunctionType

    nc = bacc.Bacc("TRN2", target_bir_lowering=False, debug=False)

    xt_d = nc.dram_tensor("xt", [D, BN], bf16, kind="ExternalInput")
    wq_d = nc.dram_tensor("wq", [128, KT_PER_D, 128], bf16, kind="ExternalInput")
    wk_d = nc.dram_tensor("wk", [128, KT_PER_D, 128], bf16, kind="ExternalInput")
    wv_d = nc.dram_tensor("wv", [128, KT_PER_D, 128], bf16, kind="ExternalInput")
    wo_d = nc.dram_tensor("wo", [128, D], bf16, kind="ExternalInput")
    bq_d = nc.dram_tensor("bq", [128, 1], f32, kind="ExternalInput")
    bk_d = nc.dram_tensor("bk", [128, 1], f32, kind="ExternalInput")
    bv_d = nc.dram_tensor("bv", [128, 1], f32, kind="ExternalInput")
    out_d = nc.dram_tensor("out", [BN, D], bf16, kind="ExternalOutput")

    with tile.TileContext(nc) as tc:
        with (
            tc.tile_pool(name="const", bufs=1) as const,
            tc.tile_pool(name="xtp", bufs=KT_PER_D) as xtp,  # bufs per col-group tag
            tc.tile_pool(name="persist", bufs=1) as persist,
            tc.tile_pool(name="avn", bufs=3) as avnp,
            tc.tile_pool(name="expp", bufs=6) as expp,
            tc.tile_pool(name="s0p", bufs=3) as s0p,
            tc.tile_pool(name="bcp", bufs=3) as bcp,
            tc.tile_pool(name="h1tp", bufs=2) as h1tp,
            tc.tile_pool(name="osb", bufs=3) as osbp,
        ):
            # ---- constants ----
            wq_sb = const.tile([128, KT_PER_D, 128], bf16, tag="wq")
            wk_sb = const.tile([128, KT_PER_D, 128], bf16, tag="wk")
            wv_sb = const.tile([128, KT_PER_D, 128], bf16, tag="wv")
            nc.sync.dma_start(wq_sb[:], wq_d[:])
            nc.sync.dma_start(wk_sb[:], wk_d[:])
            nc.sync.dma_start(wv_sb[:], wv_d[:])
            wo_sb = const.tile([128, D], bf16, tag="wo")
            nc.sync.dma_start(wo_sb[:], wo_d[:])
            bq_sb = const.tile([128, 1], f32, tag="bq")
            bk_sb = const.tile([128, 1], f32, tag="bk")
            nc.sync.dma_start(bq_sb[:], bq_d[:])
            nc.sync.dma_start(bk_sb[:], bk_d[:])
            bv_sb = const.tile([128, 1], f32, tag="bv")
            nc.sync.dma_start(bv_sb[:], bv_d[:])
            # causal keep-mask: mask[p, f] = 1.0 iff f >= p
            mask = const.tile([128, 128], bf16, tag="mask")
            make_upper_triangular(nc, mask[:], val=1.0, diag=True)

            # ---- X^T tiles, split by column-group so matmuls start on the
            # first 256 KB instead of after the full 8 MB ----
            NCG = 4
            CGW = BN // NCG  # 1024 cols per group
            xt_t = {}
            for cg in range(NCG):
                for k in range(KT_PER_D):
                    t = xtp.tile([128, CGW], bf16, tag=f"xt{cg}", name=f"xt{k}_{cg}")
                    nc.sync.dma_start(
                        t[:], xt_d[k * 128 : (k + 1) * 128, cg * CGW : (cg + 1) * CGW]
                    )
                    xt_t[(k, cg)] = t

            def xt_slice(k, c0, c1):
                cg = c0 // CGW
                assert c1 <= (cg + 1) * CGW
                return xt_t[(k, cg)][:, c0 - cg * CGW : c1 - cg * CGW]

            # ---- persistent activations ----
            QT = persist.tile([128, BN], bf16, tag="qt")
            KT = persist.tile([128, BN], bf16, tag="kt")
            VT = persist.tile([128, BN], bf16, tag="vt")
            # V blocks with interleaved ones cols: [V_h0 | 1 | V_h1 | 1]
            V_sb = persist.tile([128, NBLK_ALL, 130], bf16, tag="vsb")
            vview = V_sb.rearrange("p i (g c) -> p i g c", c=65)
            nc.vector.memset(vview[:, :, :, 64:65], 1.0)

            def body():
                # PSUM budget (8 banks): s 2x2 + av 2 + pq (proj/out shared) 2
                with (
                    tc.tile_pool(name="pp", bufs=2, space="PSUM") as pp,
                    tc.tile_pool(name="sp", bufs=2, space="PSUM") as sp,
                    tc.tile_pool(name="avp", bufs=2, space="PSUM") as avp,
                ):
                    def emit_proj_chunk(j8, pump=False, which=("q", "k", "v"), evac_act=False):
                        """Q/K/V^T projection for one 512-col chunk; as a
                        generator it yields once per full 8-matmul chain so
                        pumped fillers never get split mid-chain (split
                        chains run ~1.7x slower on HW)."""
                        c0, c1 = j8 * NQ_CHUNK, (j8 + 1) * NQ_CHUNK
                        for w_sb, b_sb, dst, nm in (
                            (wq_sb, bq_sb, QT, "q"),
                            (wk_sb, bk_sb, KT, "k"),
                            (wv_sb, bv_sb, VT, "v"),
                        ):
                            if nm not in which:
                                continue
                            ps = pp.tile(
                                [128, NQ_CHUNK], f32, tag="pq", name=f"ps{nm}{j8}"
                            )
                            for k in range(KT_PER_D):
                                nc.tensor.matmul(
                                    ps[:],
                                    w_sb[:, k, :],
                                    xt_slice(k, c0, c1),
                                    start=(k == 0),
                                    stop=(k == KT_PER_D - 1),
                                )
                            if evac_act:
                                # fill idle ACT at kernel start (before exps)
                                nc.scalar.activation(
                                    dst[:, c0:c1], ps[:], AF.Identity, bias=b_sb[:, 0:1]
                                )
                            else:
                                nc.vector.tensor_scalar_add(dst[:, c0:c1], ps[:], b_sb[:, 0:1])
                            if pump:
                                yield

                    def gen_v_chunk(j8):
                        yield from emit_proj_chunk(j8, pump=True, which=("v",))
                        emit_v_layout(0, j8)
                        yield

                    def emit_v_layout(b, q):
                        # one 512-col quarter: xbar transpose to contiguous
                        # scratch (strided 3D transpose output misbehaves on
                        # HW), then DVE-copy into the interleaved layout.
                        nb0 = b * N_KBLK + q * 4
                        for h in range(HPC):
                            vtmp = h1tp.tile(
                                [128, 4, 64], bf16, tag="vtmp", name=f"vtmp{b}_{q}_{h}"
                            )
                            nc.sync.dma_start_transpose(
                                vtmp[:],
                                VT[h * 64 : (h + 1) * 64, b * N + q * 512 : b * N + (q + 1) * 512],
                            )
                            nc.vector.tensor_copy(
                                V_sb[:, nb0 : nb0 + 4, 65 * h : 65 * h + 64], vtmp[:]
                            )

                    def gen_b1_tail():
                        for j8 in range(N_JCH, 2 * N_JCH):
                            yield from emit_proj_chunk(j8, pump=True)
                            emit_v_layout(1, j8 - N_JCH)
                            yield

                    def gen_out(b, j, AVnj):
                        """Output projection for one normalized q-chunk."""
                        for nb in range(NQ_CHUNK // 128):
                            row0 = b * N + j * NQ_CHUNK + nb * 128
                            osb = osbp.tile(
                                [128, D], bf16, tag="osb", name=f"osb{b}_{j}_{nb}"
                            )
                            for half in range(2):
                                o_ps = pp.tile(
                                    [128, 512], f32, tag="pq", name=f"o{b}_{j}_{nb}_{half}"
                                )
                                nc.tensor.matmul(
                                    o_ps[:],
                                    AVnj[:, nb * 128 : (nb + 1) * 128],
                                    wo_sb[:, half * 512 : (half + 1) * 512],
                                    start=True,
                                    stop=True,
                                )
                                # any: let the scheduler pick the engine —
                                # keeps these 64 evacs off the DVE queue,
                                # which carries the exp-mask critical path
                                nc.any.tensor_copy(
                                    osb[:, half * 512 : (half + 1) * 512], o_ps[:]
                                )
                                yield
                            nc.sync.dma_start(out_d[row0 : row0 + 128, :], osb[:])
                            yield

                    pending = []

                    def pump(n):
                        done = 0
                        while pending and done < n:
                            try:
                                next(pending[0])
                                done += 1
                            except StopIteration:
                                pending.pop(0)

                    def drain(gen):
                        """Finish emitting one specific pending generator."""
                        while gen in pending:
                            try:
                                next(gen)
                            except StopIteration:
                                pending.remove(gen)

                    b1g = gen_b1_tail()
                    pending.append(b1g)



                    def emit_S(b, j, i):
                        """Both heads' S^T for k-block i into one 2-bank psum
                        tile: h0 cols [f0,512), h1 cols [512+f0, 1024)."""
                        cb = b * N
                        q0 = cb + j * NQ_CHUNK
                        r = i - j * (NQ_CHUNK // NK_BLK)
                        f0 = 128 * r if r > 0 else 0
                        s_ps = sp.tile(
                            [128, 2 * NQ_CHUNK], f32, tag="s", name=f"s{b}_{j}_{i}"
                        )
                        for h in range(HPC):
                            nc.tensor.matmul(
                                s_ps[:, h * NQ_CHUNK + f0 : (h + 1) * NQ_CHUNK],
                                KT[h * 64 : (h + 1) * 64, cb + i * 128 : cb + (i + 1) * 128],
                                QT[h * 64 : (h + 1) * 64, q0 + f0 : q0 + NQ_CHUNK],
                                start=True,
                                stop=True,
                            )
                        return s_ps, f0, r

                    def emit_exp(b, j, i, s_ps, f0, r):
                        """One ACT exp instruction covering both heads."""
                        et = expp.tile(
                            [128, 2 * NQ_CHUNK], bf16, tag="exp", name=f"e{b}_{j}_{i}"
                        )
                        if f0 == 0:
                            nc.scalar.activation(et[:], s_ps[:], AF.Exp)
                        else:
                            sv = s_ps.rearrange("p (g c) -> p g c", c=NQ_CHUNK)
                            ev = et.rearrange("p (g c) -> p g c", c=NQ_CHUNK)
                            nc.scalar.activation(
                                ev[:, :, f0:NQ_CHUNK], sv[:, :, f0:NQ_CHUNK], AF.Exp
                            )
                        if r >= 0:
                            for h in range(HPC):
                                nc.vector.tensor_mul(
                                    et[:, h * NQ_CHUNK + f0 : h * NQ_CHUNK + f0 + 128],
                                    et[:, h * NQ_CHUNK + f0 : h * NQ_CHUNK + f0 + 128],
                                    mask[:],
                                )
                        return et

                    def emit_AV1(b, j, i, et, f0, h, av_ps, kmax):
                        nc.tensor.matmul(
                            av_ps[h][:, f0:NQ_CHUNK],
                            V_sb[:, b * N_KBLK + i, 65 * h : 65 * (h + 1)],
                            et[:, h * NQ_CHUNK + f0 : (h + 1) * NQ_CHUNK],
                            start=(i == 0),
                            stop=(i == kmax - 1),
                            skip_group_check=True,
                        )

                    if phase == "proj":
                        # projections only: all 8 chunks + V layouts
                        for j8 in range(N_JCH):
                            for _ in emit_proj_chunk(j8, pump=False,
                                                     evac_act=(j8 == 0)):
                                pass
                            emit_v_layout(0, j8)
                        pump(10**9)
                        nc.sync.dma_start(out_d[0:128, :], QT[:, 0:D])
                        return
                    for b in range(B):
                        j_order = range(N_JCH) if b == 0 else range(N_JCH - 1, -1, -1)
                        for j in j_order:
                            if b == 0:
                                # Q/K land just-in-time per chunk; V of chunk j
                                # becomes front-of-queue PE filler
                                qk = ("q", "k", "v") if j == 0 else ("q", "k")
                                for _ in emit_proj_chunk(j, pump=False, which=qk,
                                                         evac_act=(j == 0)):
                                    pass
                                if j == 0:
                                    emit_v_layout(0, 0)
                                else:
                                    # must fully emit before the i-loop: AV(i)
                                    # reads V_sb blocks this generator writes,
                                    # and a read emitted first gets no dep.
                                    # 2 extra pumps beyond the 2-step
                                    # generator fill the chunk-boundary
                                    # av-WAR window (measured faster)
                                    pending.insert(0, gen_v_chunk(j))
                                    pump(4)
                            else:
                                # same boundary treatment for b1 chunks:
                                # fill the av-WAR window with fillers
                                pump(4)
                            kmax = (j + 1) * (NQ_CHUNK // NK_BLK)
                            av_ps = [
                                avp.tile([65, NQ_CHUNK], f32, tag="av", name=f"av{b}_{j}_{h}")
                                for h in range(HPC)
                            ]
                            # pair-pipelined: emit S for blocks (2p, 2p+1),
                            # then the PREVIOUS pair's AVs ordered h-major so
                            # each head's two AVs chain in one PSUM group
                            # (measured 32% faster than block-major order)
                            prev = None
                            for p in range(kmax // 2):
                                cur = []
                                for i in (2 * p, 2 * p + 1):
                                    s_ps, f0, r = emit_S(b, j, i)
                                    cur.append((i, s_ps, f0, r))
                                if prev is not None:
                                    for h in range(HPC):
                                        for i, et, f0 in prev:
                                            emit_AV1(b, j, i, et, f0, h, av_ps, kmax)
                                prev = [
                                    (i, emit_exp(b, j, i, s_ps, f0, r), f0)
                                    for i, s_ps, f0, r in cur
                                ]
                                pump(2 if p == 0 else 1)
                            for h in range(HPC):
                                for i, et, f0 in prev:
                                    emit_AV1(b, j, i, et, f0, h, av_ps, kmax)
                            # normalize this chunk straight out of PSUM
                            AVnj = avnp.tile(
                                [128, NQ_CHUNK], bf16, tag="avn", name=f"avn{b}_{j}"
                            )
                            for h in range(HPC):
                                rc = s0p.tile(
                                    [65, NQ_CHUNK], f32, tag="rc", name=f"rc{b}_{j}_{h}"
                                )
                                nc.vector.reciprocal(rc[64:65, :], av_ps[h][64:65, :])
                                s0 = s0p.tile(
                                    [1, NQ_CHUNK], f32, tag="s0", name=f"s0_{b}_{j}_{h}"
                                )
                                nc.sync.dma_start(s0[0:1, :], rc[64:65, :])
                                bc = bcp.tile(
                                    [64, NQ_CHUNK], f32, tag="bc", name=f"bc{b}_{j}_{h}"
                                )
                                nc.gpsimd.partition_broadcast(bc[:], s0[0:1, :])
                                if h == 0:
                                    nc.vector.tensor_mul(
                                        AVnj[0:64, :], av_ps[h][0:64, :], bc[:]
                                    )
                                else:
                                    h1t = h1tp.tile(
                                        [64, NQ_CHUNK], bf16, tag="h1t", name=f"h1t{b}_{j}"
                                    )
                                    nc.vector.tensor_mul(h1t[:], av_ps[h][0:64, :], bc[:])
                                    nc.sync.dma_start(AVnj[64:128, :], h1t[:])
                            if phase == "noout":
                                nc.sync.dma_start(
                                    out_d[(b * N_JCH + j) * 128 : (b * N_JCH + j) * 128 + 128, 0:NQ_CHUNK],
                                    AVnj[:],
                                )
                            else:
                                pending.append(gen_out(b, j, AVnj))
                        if b == 0:
                            # batch-1 projections must be EMITTED before its
                            # attention reads QT/KT/V (emission-order dep
                            # safety) — but leftover out-proj fillers stay
                            # queued for b1's pair pumps instead of being
                            # dumped here as one ACT-idling PE backlog
                            drain(b1g)
                    pump(10**9)

            if iters > 1:
                with tc.For_i(0, iters, 1):
                    body()
            else:
                body()

    nc.compile()
    return nc


def _prep_in_maps(X, Wq, bq, Wk, bk, Wv, bv, Wo, bo):
    import ml_dtypes

    bf16 = ml_dtypes.bfloat16

    def _pkm(w):  # [D, 128] -> [128 partition, k, 128] tile layout
        return np.ascontiguousarray(
            w.reshape(KT_PER_D, 128, 128).transpose(1, 0, 2)
        ).astype(bf16)
    scale = np.float32(1.0 / np.sqrt(DK))
    Xf = np.asarray(X, dtype=np.float32).reshape(BN, D)
    xt = np.ascontiguousarray(Xf.T).astype(bf16)
    in_maps = []
    for c in range(NCORES):
        s = slice(c * 128, (c + 1) * 128)
        in_maps.append(
            {
                "xt": xt,
                "wq": _pkm(np.asarray(Wq, np.float32)[:, s] * scale),
                "wk": _pkm(np.asarray(Wk, np.float32)[:, s]),
                "wv": _pkm(np.asarray(Wv, np.float32)[:, s]),
                "wo": np.ascontiguousarray(np.asarray(Wo, np.float32)[s, :]).astype(bf16),
                "bq": np.ascontiguousarray(
                    (np.asarray(bq, np.float32)[s] * scale).reshape(128, 1)
                ),
                "bk": np.ascontiguousarray(np.asarray(bk, np.float32)[s].reshape(128, 1)),
                "bv": np.ascontiguousarray(np.asarray(bv, np.float32)[s].reshape(128, 1)),
            }
        )
    return in_maps


def _get_nc(iters=1, phase="full"):
    key = ("nc", iters, phase)
    if key not in _STATE:
        _STATE[key] = _build_nc(iters, phase)
    return _STATE[key]


def kernel(**inputs) -> np.ndarray:
    from concourse import bass_utils

    nc = _get_nc()
    in_maps = _prep_in_maps(**inputs)
    res = bass_utils.run_bass_kernel_spmd(nc, in_maps, core_ids=list(range(NCORES)))
    acc = np.zeros((BN, D), dtype=np.float32)
    for r in res.results:
        acc += np.asarray(r["out"], dtype=np.float32)
    acc += np.asarray(inputs["bo"], np.float32)[None, :]
    return acc.reshape(B, N, D)
